# revision 30
# baseline (speedup 1.0000x reference)
"""Single-head causal self-attention on 8 NeuronCores (data-parallel over batch).

Reference computation (per batch element b):
    Q = X @ Wq + bq; K = X @ Wk + bk; V = X @ Wv + bv        # [T, DK]
    S = Q @ K.T / sqrt(DK)  (causal masked)
    out = softmax(S) @ V                                      # [T, DK]

Design (all bf16; fp8 was tested numerically and exceeds the 2e-2 error
budget in every variant):
  - X.T arrives in 4 column-chunks of 512 (ktile-major inside a chunk so
    each chunk is one dense DMA).  Chunks are DMA'd in DESCENDING order,
    weights first, and the first chunk is split in 4 so projections
    start as early as possible.
  - PE pre-warm: HAM gates the PE clock at 1.2 GHz until ~3.4us of
    sustained activity.  Dummy matmuls on a zeroed scratch tile run
    during the DMA wait so real matmuls start at 2.4 GHz.
  - Projections per chunk: pass A [Wv|Wk] -> V.T rows 0:64 / K.T rows
    64:128; pass B [Wq|Wq] (duplicated for base-partition match).
    Drains on DVE; the Scalar (Act) engine does EXP ONLY.
  - Attention: s-tile pairs, hi t-half (blocks 2,3) descending pairs
    interleaved with the projection chunks they gate on, then lo t-half
    (blocks 0,1) ascending pairs.  Scores/PV matmuls are slot-major so
    each stationary LDWEIGHTS is reused across both blocks.  Each
    (pair, block) does one merged exp over a [128,2,512] PSUM score
    tile (20 activations total).
  - Causality: skip below-diagonal blocks, memset zero-fill + one
    tri-mask multiply per slot on the diagonal block.
  - P@V accumulates [65,512] f32 PSUM per t-block; a ones column in the
    V stationaries produces the softmax denominator (row 64).
    PSUM budget: 2 proj + 2x2 scores + 2 out = 8 banks.
  - Device output per core: [65, T]; host computes (O_unnorm / l).T.
"""

import sys

sys.path.insert(0, "/opt/trn_rl_repo")

import numpy as np
import ml_dtypes

B, T, C, DK = 8, 2048, 1024, 64
KT = C // 128            # 8 k-tiles in the contraction over C
NS = T // 128            # 16 s-tiles
NCHUNK = T // 512        # 4 chunks of 512
NP = NS // 2             # 8 s-tile pairs
SCALE = 1.0 / np.sqrt(DK)
BF16 = np.dtype(ml_dtypes.bfloat16)

_CACHE = {}


def _build():
    from concourse import bass, bacc, tile

    mybir = bass.mybir
    f32 = mybir.dt.float32
    bf16 = mybir.dt.bfloat16

    nc = bacc.Bacc(
        "TRN2", target_bir_lowering=False, debug=False, num_devices=B
    )

    xc_d = [
        nc.dram_tensor(f"xc{c}", [128, KT * 512], bf16, kind="ExternalInput")
        for c in range(NCHUNK)
    ]
    w2_d = nc.dram_tensor("w2", [128, 2 * KT * 128], bf16, kind="ExternalInput")
    bvk_d = nc.dram_tensor("bvk", [128, 1], f32, kind="ExternalInput")
    bqq_d = nc.dram_tensor("bqq", [128, 1], f32, kind="ExternalInput")
    out_d = nc.dram_tensor("out", [65, T], bf16, kind="ExternalOutput")

    # packed consts: cols 0:128 upper-tri mask, 128:192 identity (rows 0:64)
    cst_np = np.zeros((128, 192), dtype=BF16)
    cst_np[:, 0:128] = np.triu(np.ones((128, 128), dtype=np.float32)).astype(BF16)
    cst_np[0:64, 128:192] = np.eye(64, dtype=np.float32).astype(BF16)
    cst_d = nc.inline_tensor(cst_np, "cst")

    EXP = mybir.ActivationFunctionType.Exp

    def jmin(p):
        return (256 * p) // 512

    with tile.TileContext(nc) as tc:
        with tc.tile_pool(name="const", bufs=1) as cpool, \
             tc.tile_pool(name="weights", bufs=1) as wpool, \
             tc.tile_pool(name="x", bufs=1) as xpool, \
             tc.tile_pool(name="acts", bufs=1) as apool, \
             tc.tile_pool(name="et", bufs=4) as etpool, \
             tc.tile_pool(name="pst", bufs=2, space="PSUM") as pst, \
             tc.tile_pool(name="pops_hi", bufs=1, space="PSUM") as pops_hi:

            # ---- DMAs: weights lead the two fast rings (sync+scalar) ----
            w2 = wpool.tile([128, 2 * KT * 128], bf16)
            nc.gpsimd.dma_start(out=w2[:, 0:KT * 128], in_=w2_d[:, 0:KT * 128])
            nc.gpsimd.dma_start(
                out=w2[:, KT * 128:2 * KT * 128],
                in_=w2_d[:, KT * 128:2 * KT * 128],
            )
            wvk = w2[:, 0:KT * 128]
            wqq = w2[:, KT * 128:2 * KT * 128]
            cst = cpool.tile([128, 192], bf16)
            nc.gpsimd.dma_start(out=cst[:], in_=cst_d[:])
            tri = cst[:, 0:128]
            ident = cst[0:64, 128:192]
            bvk = cpool.tile([128, 1], f32)
            nc.gpsimd.dma_start(out=bvk[:], in_=bvk_d[:])
            bqq = cpool.tile([128, 1], f32)
            nc.gpsimd.dma_start(out=bqq[:], in_=bqq_d[:])

            # X chunks descending, every chunk split across both rings so
            # the rings advance in lockstep and chunks complete in order
            xs = [None] * NCHUNK
            half = KT * 512 // 2
            for c in range(NCHUNK - 1, -1, -1):
                xk = xpool.tile([128, KT * 512], bf16, tag=f"x{c}")
                nc.sync.dma_start(out=xk[:, 0:half], in_=xc_d[c][:, 0:half])
                nc.scalar.dma_start(
                    out=xk[:, half:2 * half], in_=xc_d[c][:, half:2 * half]
                )
                xs[c] = xk

            # ---- PE pre-warm: HAM gates the PE at 1.2 GHz until ~3.4us of
            # sustained activity; dummy matmuls on a zeroed scratch tile
            # bridge the DMA wait so real matmuls start at 2.4 GHz.
            warm_in = cpool.tile([128, 256], bf16, name="warm_in")
            nc.gpsimd.memset(warm_in[:], 0.0)
            for w in range(22):
                wps = pst.tile([128, 256], f32, tag="st", name="warm_ps")
                nc.tensor.matmul(
                    wps[:], warm_in[:, 0:128], warm_in[:],
                    start=True, stop=True,
                )

            # persistent activations
            vk = apool.tile([128, T], bf16, tag="vk")   # V.T 0:64 | K.T 64:128
            qq = apool.tile([128, T], bf16, tag="qq")   # Q.T duplicated
            v1 = apool.tile([128, NS * 65], bf16, tag="v1")  # [V_i | 1]
            osb = apool.tile([65, T], bf16, tag="osb")

            nc.gpsimd.memset(v1[:], 1.0)

            globals_pp = [None]

            def proj_chunk(c):
                pp = globals_pp[0]
                sl = slice(512 * c, 512 * (c + 1))
                psA = pp.tile([128, 512], f32, tag="psA", name="psA")
                psB = pp.tile([128, 512], f32, tag="psB", name="psB")
                for ps, w in ((psA, wvk), (psB, wqq)):
                    for k in range(KT // 2):
                        nc.tensor.matmul(
                            ps[:],
                            w[:, 128 * k:128 * (k + 1)],
                            xs[c][:, 512 * k:512 * (k + 1)],
                            start=(k == 0), stop=False,
                        )
                for ps, w in ((psA, wvk), (psB, wqq)):
                    for k in range(KT // 2, KT):
                        nc.tensor.matmul(
                            ps[:],
                            w[:, 128 * k:128 * (k + 1)],
                            xs[c][:, 512 * k:512 * (k + 1)],
                            start=False, stop=(k == KT - 1),
                        )
                nc.vector.tensor_scalar_add(vk[:, sl], psA[:], bvk[:])
                nc.vector.tensor_scalar_add(qq[:, sl], psB[:], bqq[:])
                for i in range(4 * c, 4 * c + 4):
                    vt = pp.tile([128, 64], bf16, tag="psB", name="vt")
                    nc.tensor.transpose(
                        vt[:], vk[0:64, 128 * i:128 * (i + 1)], ident[:]
                    )
                    nc.vector.tensor_copy(v1[:, 65 * i:65 * i + 64], vt[:])

            def attn_pair(p, half_blocks, pairs, otiles, opool):
                hbase = 512 * half_blocks[0]
                i0, i1 = 2 * p, 2 * p + 1
                ts0, ts1 = 128 * i0, 128 * i1
                jm = jmin(p)
                blocks = [b for b in half_blocks if b >= jm]
                if not blocks:
                    return
                etp = etpool.tile([128, 2, 1024], bf16, tag="et", name="etp")
                sts = {}
                # scores, slot-major (stationary K-tile reused across blocks)
                for u, it in ((0, i0), (1, i1)):
                    for b in blocks:
                        s0 = max(ts0, 512 * b)
                        o0 = s0 - 512 * b
                        if b not in sts:
                            sts[b] = pst.tile(
                                [128, 2, 512], f32, tag="st", name="st"
                            )
                        nc.tensor.matmul(
                            sts[b][:, u, o0:512],
                            vk[64:128, 128 * it:128 * (it + 1)],
                            qq[64:128, s0:512 * (b + 1)],
                            start=True, stop=True,
                        )
                # merged exp per block
                for b in blocks:
                    s0 = max(ts0, 512 * b)
                    o0 = s0 - 512 * b
                    nc.scalar.activation(
                        etp[:, :, s0 - hbase:512 * (b + 1) - hbase],
                        sts[b][:, :, o0:512], EXP, scale=SCALE,
                    )
                # causal fixups on the diagonal block
                if jm in blocks:
                    if ts0 > 512 * jm:
                        nc.gpsimd.memset(
                            etp[:, 0, 512 * jm - hbase:ts0 - hbase], 0.0
                        )
                    nc.gpsimd.memset(
                        etp[:, 1, 512 * jm - hbase:ts1 - hbase], 0.0
                    )
                    nc.vector.tensor_mul(
                        etp[:, 0, ts0 - hbase:ts0 + 128 - hbase],
                        etp[:, 0, ts0 - hbase:ts0 + 128 - hbase],
                        tri[:],
                    )
                    nc.vector.tensor_mul(
                        etp[:, 1, ts1 - hbase:ts1 + 128 - hbase],
                        etp[:, 1, ts1 - hbase:ts1 + 128 - hbase],
                        tri[:],
                    )
                # P @ [V|1], slot-major
                for b in blocks:
                    if b not in otiles:
                        otiles[b] = opool.tile(
                            [65, 512], f32, tag=f"o{b}", name=f"o{b}"
                        )
                for u, it in ((0, i0), (1, i1)):
                    for b in blocks:
                        contrib = [
                            q for q in pairs
                            if b in [x for x in half_blocks if x >= jmin(q)]
                        ]
                        eb0 = 512 * b - hbase
                        nc.tensor.matmul(
                            otiles[b][:],
                            v1[:, 65 * it:65 * it + 65],
                            etp[:, u, eb0:eb0 + 512],
                            start=(p == contrib[0] and u == 0),
                            stop=(p == contrib[-1] and u == 1),
                        )
                # drain blocks whose accumulation just finished
                for b in blocks:
                    contrib = [
                        q for q in pairs
                        if b in [x for x in half_blocks if x >= jmin(q)]
                    ]
                    if p == contrib[-1]:
                        sl = slice(512 * b, 512 * (b + 1))
                        nc.vector.tensor_copy(osb[:, sl], otiles[b][:])
                        nc.sync.dma_start(out=out_d[:, sl], in_=osb[:, sl])

            # ---- interleaved schedule ----
            # proj chunks descending, each followed by the hi-half pairs it
            # gates; after the last projection the proj PSUM banks are
            # released and reused for the lo-half output tiles so the tail
            # (hi pairs 1,0 + all lo pairs) runs as one dense region.
            hi_blocks, hi_pairs = (2, 3), list(range(NP - 1, -1, -1))
            lo_blocks, lo_pairs = (0, 1), [0, 1, 2, 3]
            hi_otiles, lo_otiles = {}, {}
            # staggered: during each later proj chunk the act engine is
            # covered by the previous segment's pending exps
            with tc.tile_pool(name="pp", bufs=1, space="PSUM") as pp:
                globals_pp[0] = pp
                proj_chunk(3)
                attn_pair(7, hi_blocks, hi_pairs, hi_otiles, pops_hi)
                attn_pair(6, hi_blocks, hi_pairs, hi_otiles, pops_hi)
                proj_chunk(2)
                attn_pair(5, hi_blocks, hi_pairs, hi_otiles, pops_hi)
                proj_chunk(1)
                attn_pair(4, hi_blocks, hi_pairs, hi_otiles, pops_hi)
                attn_pair(3, hi_blocks, hi_pairs, hi_otiles, pops_hi)
                proj_chunk(0)
                attn_pair(2, hi_blocks, hi_pairs, hi_otiles, pops_hi)
                attn_pair(1, hi_blocks, hi_pairs, hi_otiles, pops_hi)
                attn_pair(0, hi_blocks, hi_pairs, hi_otiles, pops_hi)
            with tc.tile_pool(name="pops_lo", bufs=1, space="PSUM") as pops_lo:
                for p in lo_pairs:
                    attn_pair(p, lo_blocks, lo_pairs, lo_otiles, pops_lo)

    nc.compile()
    return nc


def _get_nc():
    if "nc" not in _CACHE:
        _CACHE["nc"] = _build()
    return _CACHE["nc"]


def make_in_maps(X, Wq, bq, Wk, bk, Wv, bv):
    X = np.asarray(X, dtype=np.float32)
    Wq = np.asarray(Wq, dtype=np.float32)
    Wk = np.asarray(Wk, dtype=np.float32)
    Wv = np.asarray(Wv, dtype=np.float32)
    bq = np.asarray(bq, dtype=np.float32)
    bk = np.asarray(bk, dtype=np.float32)
    bv = np.asarray(bv, dtype=np.float32)

    wvk = np.ascontiguousarray(
        np.concatenate([Wv, Wk], axis=1).reshape(KT, 128, 128)
        .transpose(1, 0, 2).reshape(128, KT * 128)
    ).astype(BF16)
    wqq = np.ascontiguousarray(
        np.concatenate([Wq, Wq], axis=1).reshape(KT, 128, 128)
        .transpose(1, 0, 2).reshape(128, KT * 128)
    ).astype(BF16)
    w2 = np.ascontiguousarray(np.concatenate([wvk, wqq], axis=1))
    bvk = np.concatenate([bv, bk]).reshape(128, 1).astype(np.float32)
    bqq = np.concatenate([bq, bq]).reshape(128, 1).astype(np.float32)

    in_maps = []
    for b in range(B):
        xt = X[b].T.astype(BF16)          # [C, T]
        m = {"w2": w2, "bvk": bvk, "bqq": bqq}
        for c in range(NCHUNK):
            blk = xt[:, 512 * c:512 * (c + 1)]          # [1024, 512]
            m[f"xc{c}"] = np.ascontiguousarray(
                blk.reshape(KT, 128, 512).transpose(1, 0, 2).reshape(128, KT * 512)
            )
        in_maps.append(m)
    return in_maps


def kernel(X, Wq, bq, Wk, bk, Wv, bv):
    from concourse.bass_utils import run_bass_kernel_spmd

    nc = _get_nc()
    in_maps = make_in_maps(X, Wq, bq, Wk, bk, Wv, bv)
    res = run_bass_kernel_spmd(nc, in_maps, list(range(B)))

    out = np.empty((B, T, DK), dtype=np.float32)
    for b in range(B):
        r = np.asarray(res.results[b]["out"], dtype=np.float32)
        out[b] = (r[:64] / r[64:65]).T
    return out


# revision 31
# speedup vs baseline: 1.0824x; 1.0824x over previous
"""Single-head causal self-attention on 8 NeuronCores (data-parallel over batch).

Reference computation (per batch element b):
    Q = X @ Wq + bq; K = X @ Wk + bk; V = X @ Wv + bv        # [T, DK]
    S = Q @ K.T / sqrt(DK)  (causal masked)
    out = softmax(S) @ V                                      # [T, DK]

Design (all bf16; fp8 was tested numerically and exceeds the 2e-2 error
budget in every variant):
  - X.T arrives in 4 column-chunks of 512 (ktile-major inside a chunk so
    each chunk is one dense DMA).  Chunks are DMA'd in DESCENDING order,
    weights first, and the first chunk is split in 4 so projections
    start as early as possible.
  - PE pre-warm: HAM gates the PE clock at 1.2 GHz until ~3.4us of
    sustained activity.  Dummy matmuls on a zeroed scratch tile run
    during the DMA wait so real matmuls start at 2.4 GHz.
  - Projections per chunk: pass A [Wv|Wk] -> V.T rows 0:64 / K.T rows
    64:128; pass B [Wq|Wq] (duplicated for base-partition match).
    Drains on DVE; the Scalar (Act) engine does EXP ONLY.
  - Attention: s-tile pairs, hi t-half (blocks 2,3) descending pairs
    interleaved with the projection chunks they gate on, then lo t-half
    (blocks 0,1) ascending pairs.  Scores/PV matmuls are slot-major so
    each stationary LDWEIGHTS is reused across both blocks.  Each
    (pair, block) does one merged exp over a [128,2,512] PSUM score
    tile (20 activations total).
  - Causality: skip below-diagonal blocks, memset zero-fill + one
    tri-mask multiply per slot on the diagonal block.
  - P@V accumulates [65,512] f32 PSUM per t-block; a ones column in the
    V stationaries produces the softmax denominator (row 64).
    PSUM budget: 2 proj + 2x2 scores + 2 out = 8 banks.
  - Device output per core: [65, T]; host computes (O_unnorm / l).T.
"""

import sys

sys.path.insert(0, "/opt/trn_rl_repo")

import numpy as np
import ml_dtypes

B, T, C, DK = 8, 2048, 1024, 64
KT = C // 128            # 8 k-tiles in the contraction over C
NS = T // 128            # 16 s-tiles
NCHUNK = T // 512        # 4 chunks of 512
NP = NS // 2             # 8 s-tile pairs
SCALE = 1.0 / np.sqrt(DK)
BF16 = np.dtype(ml_dtypes.bfloat16)

_CACHE = {}


def _build():
    from concourse import bass, bacc, tile

    mybir = bass.mybir
    f32 = mybir.dt.float32
    bf16 = mybir.dt.bfloat16

    nc = bacc.Bacc(
        "TRN2", target_bir_lowering=False, debug=False, num_devices=B
    )

    xc_d = [
        nc.dram_tensor(f"xc{c}", [128, KT * 512], bf16, kind="ExternalInput")
        for c in range(NCHUNK)
    ]
    w2_d = nc.dram_tensor("w2", [128, 2 * KT * 128], bf16, kind="ExternalInput")
    bvk_d = nc.dram_tensor("bvk", [128, 1], f32, kind="ExternalInput")
    bqq_d = nc.dram_tensor("bqq", [128, 1], f32, kind="ExternalInput")
    out_d = nc.dram_tensor("out", [65, T], bf16, kind="ExternalOutput")

    # packed consts: cols 0:128 upper-tri mask, 128:192 identity (rows 0:64)
    cst_np = np.zeros((128, 192), dtype=BF16)
    cst_np[:, 0:128] = np.triu(np.ones((128, 128), dtype=np.float32)).astype(BF16)
    cst_np[0:64, 128:192] = np.eye(64, dtype=np.float32).astype(BF16)
    cst_d = nc.inline_tensor(cst_np, "cst")

    EXP = mybir.ActivationFunctionType.Exp

    def jmin(p):
        return (256 * p) // 512

    with tile.TileContext(nc) as tc:
        with tc.tile_pool(name="const", bufs=1) as cpool, \
             tc.tile_pool(name="weights", bufs=1) as wpool, \
             tc.tile_pool(name="x", bufs=1) as xpool, \
             tc.tile_pool(name="acts", bufs=1) as apool, \
             tc.tile_pool(name="et", bufs=4) as etpool, \
             tc.tile_pool(name="pst", bufs=2, space="PSUM") as pst, \
             tc.tile_pool(name="pops_hi", bufs=1, space="PSUM") as pops_hi:

            # ---- DMAs: weights lead the two fast rings (sync+scalar) ----
            w2 = wpool.tile([128, 2 * KT * 128], bf16)
            nc.sync.dma_start(out=w2[:, 0:KT * 128], in_=w2_d[:, 0:KT * 128])
            nc.scalar.dma_start(
                out=w2[:, KT * 128:2 * KT * 128],
                in_=w2_d[:, KT * 128:2 * KT * 128],
            )
            wvk = w2[:, 0:KT * 128]
            wqq = w2[:, KT * 128:2 * KT * 128]
            cst = cpool.tile([128, 192], bf16)
            nc.gpsimd.dma_start(out=cst[:], in_=cst_d[:])
            tri = cst[:, 0:128]
            ident = cst[0:64, 128:192]
            bvk = cpool.tile([128, 1], f32)
            nc.gpsimd.dma_start(out=bvk[:], in_=bvk_d[:])
            bqq = cpool.tile([128, 1], f32)
            nc.gpsimd.dma_start(out=bqq[:], in_=bqq_d[:])

            # X chunks descending, every chunk split across both rings so
            # the rings advance in lockstep and chunks complete in order
            xs = [None] * NCHUNK
            half = KT * 512 // 2
            for c in range(NCHUNK - 1, -1, -1):
                xk = xpool.tile([128, KT * 512], bf16, tag=f"x{c}")
                nc.sync.dma_start(out=xk[:, 0:half], in_=xc_d[c][:, 0:half])
                nc.scalar.dma_start(
                    out=xk[:, half:2 * half], in_=xc_d[c][:, half:2 * half]
                )
                xs[c] = xk

            # ---- PE pre-warm: HAM gates the PE at 1.2 GHz until ~3.4us of
            # sustained activity; dummy matmuls on a zeroed scratch tile
            # bridge the DMA wait so real matmuls start at 2.4 GHz.
            warm_in = cpool.tile([128, 256], bf16, name="warm_in")
            nc.gpsimd.memset(warm_in[:], 0.0)
            for w in range(22):
                wps = pst.tile([128, 256], f32, tag="st", name="warm_ps")
                nc.tensor.matmul(
                    wps[:], warm_in[:, 0:128], warm_in[:],
                    start=True, stop=True,
                )

            # persistent activations
            vk = apool.tile([128, T], bf16, tag="vk")   # V.T 0:64 | K.T 64:128
            qq = apool.tile([128, T], bf16, tag="qq")   # Q.T duplicated
            v1 = apool.tile([128, NS * 65], bf16, tag="v1")  # [V_i | 1]
            osb = apool.tile([65, T], bf16, tag="osb")

            nc.gpsimd.memset(v1[:], 1.0)

            globals_pp = [None]

            def proj_chunk(c):
                pp = globals_pp[0]
                sl = slice(512 * c, 512 * (c + 1))
                psA = pp.tile([128, 512], f32, tag="psA", name="psA")
                psB = pp.tile([128, 512], f32, tag="psB", name="psB")
                for ps, w in ((psA, wvk), (psB, wqq)):
                    for k in range(KT // 2):
                        nc.tensor.matmul(
                            ps[:],
                            w[:, 128 * k:128 * (k + 1)],
                            xs[c][:, 512 * k:512 * (k + 1)],
                            start=(k == 0), stop=False,
                        )
                for ps, w in ((psA, wvk), (psB, wqq)):
                    for k in range(KT // 2, KT):
                        nc.tensor.matmul(
                            ps[:],
                            w[:, 128 * k:128 * (k + 1)],
                            xs[c][:, 512 * k:512 * (k + 1)],
                            start=False, stop=(k == KT - 1),
                        )
                nc.vector.tensor_scalar_add(vk[:, sl], psA[:], bvk[:])
                nc.vector.tensor_scalar_add(qq[:, sl], psB[:], bqq[:])
                for i in range(4 * c, 4 * c + 4):
                    vt = pp.tile([128, 64], bf16, tag="psB", name="vt")
                    nc.tensor.transpose(
                        vt[:], vk[0:64, 128 * i:128 * (i + 1)], ident[:]
                    )
                    nc.vector.tensor_copy(v1[:, 65 * i:65 * i + 64], vt[:])

            def attn_pair(p, half_blocks, pairs, otiles, opool):
                hbase = 512 * half_blocks[0]
                i0, i1 = 2 * p, 2 * p + 1
                ts0, ts1 = 128 * i0, 128 * i1
                jm = jmin(p)
                blocks = [b for b in half_blocks if b >= jm]
                if not blocks:
                    return
                etp = etpool.tile([128, 2, 1024], bf16, tag="et", name="etp")
                sts = {}
                # scores, slot-major (stationary K-tile reused across blocks)
                for u, it in ((0, i0), (1, i1)):
                    for b in blocks:
                        s0 = max(ts0, 512 * b)
                        o0 = s0 - 512 * b
                        if b not in sts:
                            sts[b] = pst.tile(
                                [128, 2, 512], f32, tag="st", name="st"
                            )
                        nc.tensor.matmul(
                            sts[b][:, u, o0:512],
                            vk[64:128, 128 * it:128 * (it + 1)],
                            qq[64:128, s0:512 * (b + 1)],
                            start=True, stop=True,
                        )
                # merged exp per block
                for b in blocks:
                    s0 = max(ts0, 512 * b)
                    o0 = s0 - 512 * b
                    nc.scalar.activation(
                        etp[:, :, s0 - hbase:512 * (b + 1) - hbase],
                        sts[b][:, :, o0:512], EXP, scale=SCALE,
                    )
                # causal fixups on the diagonal block
                if jm in blocks:
                    if ts0 > 512 * jm:
                        nc.gpsimd.memset(
                            etp[:, 0, 512 * jm - hbase:ts0 - hbase], 0.0
                        )
                    nc.gpsimd.memset(
                        etp[:, 1, 512 * jm - hbase:ts1 - hbase], 0.0
                    )
                    nc.vector.tensor_mul(
                        etp[:, 0, ts0 - hbase:ts0 + 128 - hbase],
                        etp[:, 0, ts0 - hbase:ts0 + 128 - hbase],
                        tri[:],
                    )
                    nc.vector.tensor_mul(
                        etp[:, 1, ts1 - hbase:ts1 + 128 - hbase],
                        etp[:, 1, ts1 - hbase:ts1 + 128 - hbase],
                        tri[:],
                    )
                # P @ [V|1], slot-major
                for b in blocks:
                    if b not in otiles:
                        otiles[b] = opool.tile(
                            [65, 512], f32, tag=f"o{b}", name=f"o{b}"
                        )
                for u, it in ((0, i0), (1, i1)):
                    for b in blocks:
                        contrib = [
                            q for q in pairs
                            if b in [x for x in half_blocks if x >= jmin(q)]
                        ]
                        eb0 = 512 * b - hbase
                        nc.tensor.matmul(
                            otiles[b][:],
                            v1[:, 65 * it:65 * it + 65],
                            etp[:, u, eb0:eb0 + 512],
                            start=(p == contrib[0] and u == 0),
                            stop=(p == contrib[-1] and u == 1),
                        )
                # drain blocks whose accumulation just finished
                for b in blocks:
                    contrib = [
                        q for q in pairs
                        if b in [x for x in half_blocks if x >= jmin(q)]
                    ]
                    if p == contrib[-1]:
                        sl = slice(512 * b, 512 * (b + 1))
                        nc.vector.tensor_copy(osb[:, sl], otiles[b][:])
                        nc.sync.dma_start(out=out_d[:, sl], in_=osb[:, sl])

            # ---- interleaved schedule ----
            # proj chunks descending, each followed by the hi-half pairs it
            # gates; after the last projection the proj PSUM banks are
            # released and reused for the lo-half output tiles so the tail
            # (hi pairs 1,0 + all lo pairs) runs as one dense region.
            hi_blocks, hi_pairs = (2, 3), list(range(NP - 1, -1, -1))
            lo_blocks, lo_pairs = (0, 1), [0, 1, 2, 3]
            hi_otiles, lo_otiles = {}, {}
            # staggered: during each later proj chunk the act engine is
            # covered by the previous segment's pending exps
            with tc.tile_pool(name="pp", bufs=1, space="PSUM") as pp:
                globals_pp[0] = pp
                proj_chunk(3)
                attn_pair(7, hi_blocks, hi_pairs, hi_otiles, pops_hi)
                attn_pair(6, hi_blocks, hi_pairs, hi_otiles, pops_hi)
                proj_chunk(2)
                attn_pair(5, hi_blocks, hi_pairs, hi_otiles, pops_hi)
                proj_chunk(1)
                attn_pair(4, hi_blocks, hi_pairs, hi_otiles, pops_hi)
                attn_pair(3, hi_blocks, hi_pairs, hi_otiles, pops_hi)
                proj_chunk(0)
                attn_pair(2, hi_blocks, hi_pairs, hi_otiles, pops_hi)
                attn_pair(1, hi_blocks, hi_pairs, hi_otiles, pops_hi)
                attn_pair(0, hi_blocks, hi_pairs, hi_otiles, pops_hi)
            with tc.tile_pool(name="pops_lo", bufs=1, space="PSUM") as pops_lo:
                for p in lo_pairs:
                    attn_pair(p, lo_blocks, lo_pairs, lo_otiles, pops_lo)

    nc.compile()
    return nc


def _get_nc():
    if "nc" not in _CACHE:
        _CACHE["nc"] = _build()
    return _CACHE["nc"]


def make_in_maps(X, Wq, bq, Wk, bk, Wv, bv):
    X = np.asarray(X, dtype=np.float32)
    Wq = np.asarray(Wq, dtype=np.float32)
    Wk = np.asarray(Wk, dtype=np.float32)
    Wv = np.asarray(Wv, dtype=np.float32)
    bq = np.asarray(bq, dtype=np.float32)
    bk = np.asarray(bk, dtype=np.float32)
    bv = np.asarray(bv, dtype=np.float32)

    wvk = np.ascontiguousarray(
        np.concatenate([Wv, Wk], axis=1).reshape(KT, 128, 128)
        .transpose(1, 0, 2).reshape(128, KT * 128)
    ).astype(BF16)
    wqq = np.ascontiguousarray(
        np.concatenate([Wq, Wq], axis=1).reshape(KT, 128, 128)
        .transpose(1, 0, 2).reshape(128, KT * 128)
    ).astype(BF16)
    w2 = np.ascontiguousarray(np.concatenate([wvk, wqq], axis=1))
    bvk = np.concatenate([bv, bk]).reshape(128, 1).astype(np.float32)
    bqq = np.concatenate([bq, bq]).reshape(128, 1).astype(np.float32)

    in_maps = []
    for b in range(B):
        xt = X[b].T.astype(BF16)          # [C, T]
        m = {"w2": w2, "bvk": bvk, "bqq": bqq}
        for c in range(NCHUNK):
            blk = xt[:, 512 * c:512 * (c + 1)]          # [1024, 512]
            m[f"xc{c}"] = np.ascontiguousarray(
                blk.reshape(KT, 128, 512).transpose(1, 0, 2).reshape(128, KT * 512)
            )
        in_maps.append(m)
    return in_maps


def kernel(X, Wq, bq, Wk, bk, Wv, bv):
    from concourse.bass_utils import run_bass_kernel_spmd

    nc = _get_nc()
    in_maps = make_in_maps(X, Wq, bq, Wk, bk, Wv, bv)
    res = run_bass_kernel_spmd(nc, in_maps, list(range(B)))

    out = np.empty((B, T, DK), dtype=np.float32)
    for b in range(B):
        r = np.asarray(res.results[b]["out"], dtype=np.float32)
        out[b] = (r[:64] / r[64:65]).T
    return out


# revision 32
# speedup vs baseline: 1.1020x; 1.0182x over previous
"""Single-head causal self-attention on 8 NeuronCores (data-parallel over batch).

Reference computation (per batch element b):
    Q = X @ Wq + bq; K = X @ Wk + bk; V = X @ Wv + bv        # [T, DK]
    S = Q @ K.T / sqrt(DK)  (causal masked)
    out = softmax(S) @ V                                      # [T, DK]

Design (all bf16; fp8 was tested numerically and exceeds the 2e-2 error
budget in every variant, so the wins are scheduling, not dtype):
  - X.T arrives in 4 column-chunks of 512 (ktile-major inside a chunk).
    Chunks are DMA'd in DESCENDING order, each split across the two fast
    DMA rings (sync+scalar) with the packed weights leading both rings,
    so chunk 3 + weights land ~6us after DMA start and later chunks
    arrive in lockstep with compute.
  - PE pre-warm: the HAM activity monitor gates the PE clock at 1.2 GHz
    until ~3.4us of sustained activity.  Dummy matmuls on a zeroed
    scratch tile bridge the DMA wait so real matmuls start at 2.4 GHz.
  - Projections per chunk: pass A [Wv|Wk] -> V.T rows 0:64 / K.T rows
    64:128; pass B [Wq|Wq] (duplicated for base-partition match with K
    in the scores matmul), interleaved at half-k granularity.  PSUM
    drains on DVE; the Scalar (Act) engine does EXP ONLY.
  - Attention: s-tile pairs.  Hi t-half (blocks 2,3) in descending pair
    order interleaved ("staggered") with the projection chunks so the
    act engine always has pending exps while the PE projects; then lo
    t-half (blocks 0,1) in ascending order, reusing the projection PSUM
    banks, so block 0 drains early and the kernel ends on a minimal
    chain.  Scores/PV matmuls are slot-major so each stationary
    LDWEIGHTS is reused across both blocks of a pair.  Each
    (pair, block) does ONE merged exp over a [128,2,512] PSUM score
    tile covering both s-tiles (20 activations total).
  - Causality: descending/ascending pair order skips below-diagonal
    blocks; the diagonal block gets memset zero-fill + one tri-mask
    multiply per slot after the exp.
  - P@V accumulates [65,512] f32 PSUM per t-block; a ones column in the
    V stationaries produces the softmax denominator (row 64).
    PSUM budget: 2 proj + 2x2 scores + 2+2 out = 8 banks.
  - Device output per core: [65, T]; host computes (O_unnorm / l).T.

Measured on hardware: ~60-67us vs the 85.9us baseline (HW exec time via
neuron-profile; run-to-run spread is HAM clock-gate phase luck).
"""

import sys

sys.path.insert(0, "/opt/trn_rl_repo")

import numpy as np
import ml_dtypes

B, T, C, DK = 8, 2048, 1024, 64
KT = C // 128            # 8 k-tiles in the contraction over C
NS = T // 128            # 16 s-tiles
NCHUNK = T // 512        # 4 chunks of 512
NP = NS // 2             # 8 s-tile pairs
SCALE = 1.0 / np.sqrt(DK)
BF16 = np.dtype(ml_dtypes.bfloat16)

_CACHE = {}


def _build():
    from concourse import bass, bacc, tile

    mybir = bass.mybir
    f32 = mybir.dt.float32
    bf16 = mybir.dt.bfloat16

    nc = bacc.Bacc(
        "TRN2", target_bir_lowering=False, debug=False, num_devices=B
    )

    xc_d = [
        nc.dram_tensor(f"xc{c}", [128, KT * 512], bf16, kind="ExternalInput")
        for c in range(NCHUNK)
    ]
    w2_d = nc.dram_tensor("w2", [128, 2 * KT * 128], bf16, kind="ExternalInput")
    bvk_d = nc.dram_tensor("bvk", [128, 1], f32, kind="ExternalInput")
    bqq_d = nc.dram_tensor("bqq", [128, 1], f32, kind="ExternalInput")
    out_d = nc.dram_tensor("out", [65, T], bf16, kind="ExternalOutput")

    # packed consts: cols 0:128 upper-tri mask, 128:192 identity (rows 0:64)
    cst_np = np.zeros((128, 192), dtype=BF16)
    cst_np[:, 0:128] = np.triu(np.ones((128, 128), dtype=np.float32)).astype(BF16)
    cst_np[0:64, 128:192] = np.eye(64, dtype=np.float32).astype(BF16)
    cst_d = nc.inline_tensor(cst_np, "cst")

    EXP = mybir.ActivationFunctionType.Exp

    def jmin(p):
        return (256 * p) // 512

    with tile.TileContext(nc) as tc:
        with tc.tile_pool(name="const", bufs=1) as cpool, \
             tc.tile_pool(name="weights", bufs=1) as wpool, \
             tc.tile_pool(name="x", bufs=1) as xpool, \
             tc.tile_pool(name="acts", bufs=1) as apool, \
             tc.tile_pool(name="et", bufs=4) as etpool, \
             tc.tile_pool(name="pst", bufs=2, space="PSUM") as pst, \
             tc.tile_pool(name="pops_hi", bufs=1, space="PSUM") as pops_hi:

            # ---- DMAs: weights lead the two fast rings (sync+scalar) ----
            w2 = wpool.tile([128, 2 * KT * 128], bf16)
            nc.sync.dma_start(out=w2[:, 0:KT * 128], in_=w2_d[:, 0:KT * 128])
            nc.scalar.dma_start(
                out=w2[:, KT * 128:2 * KT * 128],
                in_=w2_d[:, KT * 128:2 * KT * 128],
            )
            wvk = w2[:, 0:KT * 128]
            wqq = w2[:, KT * 128:2 * KT * 128]
            cst = cpool.tile([128, 192], bf16)
            nc.gpsimd.dma_start(out=cst[:], in_=cst_d[:])
            tri = cst[:, 0:128]
            ident = cst[0:64, 128:192]
            bvk = cpool.tile([128, 1], f32)
            nc.gpsimd.dma_start(out=bvk[:], in_=bvk_d[:])
            bqq = cpool.tile([128, 1], f32)
            nc.gpsimd.dma_start(out=bqq[:], in_=bqq_d[:])

            # X chunks descending, every chunk split across both rings so
            # the rings advance in lockstep and chunks complete in order
            xs = [None] * NCHUNK
            half = KT * 512 // 2
            for c in range(NCHUNK - 1, -1, -1):
                xk = xpool.tile([128, KT * 512], bf16, tag=f"x{c}")
                nc.sync.dma_start(out=xk[:, 0:half], in_=xc_d[c][:, 0:half])
                nc.scalar.dma_start(
                    out=xk[:, half:2 * half], in_=xc_d[c][:, half:2 * half]
                )
                xs[c] = xk

            # ---- PE pre-warm: HAM gates the PE at 1.2 GHz until ~3.4us of
            # sustained activity; dummy matmuls on a zeroed scratch tile
            # bridge the DMA wait so real matmuls start at 2.4 GHz.
            warm_in = cpool.tile([128, 256], bf16, name="warm_in")
            nc.gpsimd.memset(warm_in[:], 0.0)
            for w in range(22):
                wps = pst.tile([128, 256], f32, tag="st", name="warm_ps")
                nc.tensor.matmul(
                    wps[:], warm_in[:, 0:128], warm_in[:],
                    start=True, stop=True,
                )

            # persistent activations
            vk = apool.tile([128, T], bf16, tag="vk")   # V.T 0:64 | K.T 64:128
            qq = apool.tile([128, T], bf16, tag="qq")   # Q.T duplicated
            v1 = apool.tile([128, NS * 65], bf16, tag="v1")  # [V_i | 1]
            osb = apool.tile([65, T], bf16, tag="osb")

            nc.gpsimd.memset(v1[:], 1.0)

            globals_pp = [None]

            def proj_chunk(c):
                pp = globals_pp[0]
                sl = slice(512 * c, 512 * (c + 1))
                psA = pp.tile([128, 512], f32, tag="psA", name="psA")
                psB = pp.tile([128, 512], f32, tag="psB", name="psB")
                for ps, w in ((psA, wvk), (psB, wqq)):
                    for k in range(KT // 2):
                        nc.tensor.matmul(
                            ps[:],
                            w[:, 128 * k:128 * (k + 1)],
                            xs[c][:, 512 * k:512 * (k + 1)],
                            start=(k == 0), stop=False,
                        )
                for ps, w in ((psA, wvk), (psB, wqq)):
                    for k in range(KT // 2, KT):
                        nc.tensor.matmul(
                            ps[:],
                            w[:, 128 * k:128 * (k + 1)],
                            xs[c][:, 512 * k:512 * (k + 1)],
                            start=False, stop=(k == KT - 1),
                        )
                nc.vector.tensor_scalar_add(vk[:, sl], psA[:], bvk[:])
                nc.vector.tensor_scalar_add(qq[:, sl], psB[:], bqq[:])
                for i in range(4 * c, 4 * c + 4):
                    vt = pp.tile([128, 64], bf16, tag="psB", name="vt")
                    nc.tensor.transpose(
                        vt[:], vk[0:64, 128 * i:128 * (i + 1)], ident[:]
                    )
                    nc.vector.tensor_copy(v1[:, 65 * i:65 * i + 64], vt[:])

            def attn_pair(p, half_blocks, pairs, otiles, opool):
                hbase = 512 * half_blocks[0]
                i0, i1 = 2 * p, 2 * p + 1
                ts0, ts1 = 128 * i0, 128 * i1
                jm = jmin(p)
                blocks = [b for b in half_blocks if b >= jm]
                if not blocks:
                    return
                etp = etpool.tile([128, 2, 1024], bf16, tag="et", name="etp")
                sts = {}
                # scores, slot-major (stationary K-tile reused across blocks)
                for u, it in ((0, i0), (1, i1)):
                    for b in blocks:
                        s0 = max(ts0, 512 * b)
                        o0 = s0 - 512 * b
                        if b not in sts:
                            sts[b] = pst.tile(
                                [128, 2, 512], f32, tag="st", name="st"
                            )
                        nc.tensor.matmul(
                            sts[b][:, u, o0:512],
                            vk[64:128, 128 * it:128 * (it + 1)],
                            qq[64:128, s0:512 * (b + 1)],
                            start=True, stop=True,
                        )
                # merged exp per block
                for b in blocks:
                    s0 = max(ts0, 512 * b)
                    o0 = s0 - 512 * b
                    nc.scalar.activation(
                        etp[:, :, s0 - hbase:512 * (b + 1) - hbase],
                        sts[b][:, :, o0:512], EXP, scale=SCALE,
                    )
                # causal fixups on the diagonal block
                if jm in blocks:
                    if ts0 > 512 * jm:
                        nc.gpsimd.memset(
                            etp[:, 0, 512 * jm - hbase:ts0 - hbase], 0.0
                        )
                    nc.gpsimd.memset(
                        etp[:, 1, 512 * jm - hbase:ts1 - hbase], 0.0
                    )
                    nc.vector.tensor_mul(
                        etp[:, 0, ts0 - hbase:ts0 + 128 - hbase],
                        etp[:, 0, ts0 - hbase:ts0 + 128 - hbase],
                        tri[:],
                    )
                    nc.vector.tensor_mul(
                        etp[:, 1, ts1 - hbase:ts1 + 128 - hbase],
                        etp[:, 1, ts1 - hbase:ts1 + 128 - hbase],
                        tri[:],
                    )
                # P @ [V|1], slot-major
                for b in blocks:
                    if b not in otiles:
                        otiles[b] = opool.tile(
                            [65, 512], f32, tag=f"o{b}", name=f"o{b}"
                        )
                for u, it in ((0, i0), (1, i1)):
                    for b in blocks:
                        contrib = [
                            q for q in pairs
                            if b in [x for x in half_blocks if x >= jmin(q)]
                        ]
                        eb0 = 512 * b - hbase
                        nc.tensor.matmul(
                            otiles[b][:],
                            v1[:, 65 * it:65 * it + 65],
                            etp[:, u, eb0:eb0 + 512],
                            start=(p == contrib[0] and u == 0),
                            stop=(p == contrib[-1] and u == 1),
                        )
                # drain blocks whose accumulation just finished
                for b in blocks:
                    contrib = [
                        q for q in pairs
                        if b in [x for x in half_blocks if x >= jmin(q)]
                    ]
                    if p == contrib[-1]:
                        sl = slice(512 * b, 512 * (b + 1))
                        nc.vector.tensor_copy(osb[:, sl], otiles[b][:])
                        nc.sync.dma_start(out=out_d[:, sl], in_=osb[:, sl])

            # ---- interleaved schedule ----
            # proj chunks descending, each followed by the hi-half pairs it
            # gates; after the last projection the proj PSUM banks are
            # released and reused for the lo-half output tiles so the tail
            # (hi pairs 1,0 + all lo pairs) runs as one dense region.
            hi_blocks, hi_pairs = (2, 3), list(range(NP - 1, -1, -1))
            lo_blocks, lo_pairs = (0, 1), [0, 1, 2, 3]
            hi_otiles, lo_otiles = {}, {}
            # staggered: during each later proj chunk the act engine is
            # covered by the previous segment's pending exps
            with tc.tile_pool(name="pp", bufs=1, space="PSUM") as pp:
                globals_pp[0] = pp
                proj_chunk(3)
                attn_pair(7, hi_blocks, hi_pairs, hi_otiles, pops_hi)
                attn_pair(6, hi_blocks, hi_pairs, hi_otiles, pops_hi)
                proj_chunk(2)
                attn_pair(5, hi_blocks, hi_pairs, hi_otiles, pops_hi)
                proj_chunk(1)
                attn_pair(4, hi_blocks, hi_pairs, hi_otiles, pops_hi)
                attn_pair(3, hi_blocks, hi_pairs, hi_otiles, pops_hi)
                proj_chunk(0)
                attn_pair(2, hi_blocks, hi_pairs, hi_otiles, pops_hi)
                attn_pair(1, hi_blocks, hi_pairs, hi_otiles, pops_hi)
                attn_pair(0, hi_blocks, hi_pairs, hi_otiles, pops_hi)
            with tc.tile_pool(name="pops_lo", bufs=1, space="PSUM") as pops_lo:
                for p in lo_pairs:
                    attn_pair(p, lo_blocks, lo_pairs, lo_otiles, pops_lo)

    nc.compile()
    return nc


def _get_nc():
    if "nc" not in _CACHE:
        _CACHE["nc"] = _build()
    return _CACHE["nc"]


def make_in_maps(X, Wq, bq, Wk, bk, Wv, bv):
    X = np.asarray(X, dtype=np.float32)
    Wq = np.asarray(Wq, dtype=np.float32)
    Wk = np.asarray(Wk, dtype=np.float32)
    Wv = np.asarray(Wv, dtype=np.float32)
    bq = np.asarray(bq, dtype=np.float32)
    bk = np.asarray(bk, dtype=np.float32)
    bv = np.asarray(bv, dtype=np.float32)

    wvk = np.ascontiguousarray(
        np.concatenate([Wv, Wk], axis=1).reshape(KT, 128, 128)
        .transpose(1, 0, 2).reshape(128, KT * 128)
    ).astype(BF16)
    wqq = np.ascontiguousarray(
        np.concatenate([Wq, Wq], axis=1).reshape(KT, 128, 128)
        .transpose(1, 0, 2).reshape(128, KT * 128)
    ).astype(BF16)
    w2 = np.ascontiguousarray(np.concatenate([wvk, wqq], axis=1))
    bvk = np.concatenate([bv, bk]).reshape(128, 1).astype(np.float32)
    bqq = np.concatenate([bq, bq]).reshape(128, 1).astype(np.float32)

    in_maps = []
    for b in range(B):
        xt = X[b].T.astype(BF16)          # [C, T]
        m = {"w2": w2, "bvk": bvk, "bqq": bqq}
        for c in range(NCHUNK):
            blk = xt[:, 512 * c:512 * (c + 1)]          # [1024, 512]
            m[f"xc{c}"] = np.ascontiguousarray(
                blk.reshape(KT, 128, 512).transpose(1, 0, 2).reshape(128, KT * 512)
            )
        in_maps.append(m)
    return in_maps


def kernel(X, Wq, bq, Wk, bk, Wv, bv):
    from concourse.bass_utils import run_bass_kernel_spmd

    nc = _get_nc()
    in_maps = make_in_maps(X, Wq, bq, Wk, bk, Wv, bv)
    res = run_bass_kernel_spmd(nc, in_maps, list(range(B)))

    out = np.empty((B, T, DK), dtype=np.float32)
    for b in range(B):
        r = np.asarray(res.results[b]["out"], dtype=np.float32)
        out[b] = (r[:64] / r[64:65]).T
    return out
